# revision 1
# baseline (speedup 1.0000x reference)
"""Multi-head self-attention (B=4, S=4096, D=128, H=4, no scaling, no mask)
on 8 Trainium2 NeuronCores.

Sharding: 16 (batch, head) pairs over 8 cores -> core c handles batch c//2,
heads 2*(c%2) and 2*(c%2)+1. No cross-core communication.

Per-core algorithm (flash-style, scores never touch DRAM), v2:
  - 2-chunk score groups: psum scores tile [128, 1024] covers 2 key-chunks
    of 128 keys x 512 queries, via 2 row-tiled matmuls (tile_position
    (32r,0), 2-replicated q / packed kT).
  - software pipeline with lookahead 2 (pss bufs=3): PE never waits on the
    exp latency; PV(g) runs 2 group-slots after its scores.
  - exp split across engines: per block, 11 groups use ACT Exp, 5 groups
    use a DVE Schraudolph fast-exp (int32(A*s+B) bitcast to f32; ~3%
    sawtooth error, within the 2e-2 tolerance; denominators stay
    consistent because the ones-column sums the same approximated values).
  - bk is dropped entirely: softmax over keys is invariant to the
    per-query constant (q+bq)@bk, so k = x@Wk.T reproduces the reference
    softmax exactly.
  - av accumulator [33,512] psum double-buffered (psav bufs=2) -> no PE
    stall at block boundaries; av evacuation + normalization on DVE.
  - no separate projection psum pool: projection matmuls allocate from the
    scores pool (pss). xt arrives in 4 tiles of 1024 cols; tile c's
    projections are interleaved into block 0 (pipeline drained at those 3
    points), h1 projections trickle in one at a time at later block
    boundaries where exactly one psum buffer is free.
Host gathers OUT [2, 32, S] per core into the full (B, S, D) output.
"""

import sys

for _p in ("/opt/trn_rl_repo", "/root/.axon_site/_ro/trn_rl_repo"):
    if _p not in sys.path:
        sys.path.append(_p)

import numpy as np
from collections import deque
from contextlib import ExitStack

import concourse.bass as bass
import concourse.bacc as bacc
import concourse.mybir as mybir
import concourse.tile as tile
from concourse import bass_utils

F32 = mybir.dt.float32
F32R = mybir.dt.float32r
I32 = mybir.dt.int32
I16 = mybir.dt.int16
BF16 = mybir.dt.bfloat16
AF = mybir.ActivationFunctionType
ALU = mybir.AluOpType

B, D, H, HD = 4, 128, 4, 32
NCORES = 8

# Schraudolph fast-exp in bf16 bit-space: exp(x) ~= bitcast_bf16(int16(A*x+B))
# (bf16 = top 16 bits of f32, so the fp32 constants scale by 2^-16)
LOG2E = 1.4426950408889634
SCH_A = float(np.float32(2.0**7 * LOG2E))
SCH_C = 486411.0 / 2.0**16
SCH_B = float(np.float32(127.0 * 2.0**7 - SCH_C))

# groups (of 2 key-chunks) per block handled by the DVE fast-exp
DVE_GROUPS = frozenset({2, 4, 7, 9, 12, 14})

_built = {}


def build_nc(S):
    """Build + compile the per-core program (identical across cores)."""
    NJ = S // 128   # 128-key chunks
    NG = NJ // 2    # 2-chunk groups per block
    NQB = S // 512  # 512-query blocks per head
    NT = S // 1024  # xt DMA tiles

    nc = bacc.Bacc("TRN2", target_bir_lowering=False, debug=False)

    XT = nc.dram_tensor("XT", [128, S], F32, kind="ExternalInput").ap()
    WBLOB = nc.dram_tensor("WBLOB", [128, 518], F32, kind="ExternalInput").ap()
    OUT = nc.dram_tensor("OUT", [2, 32, S], F32, kind="ExternalOutput").ap()
    # WBLOB cols: 0:128 wq (2-replicated), 128:384 wk (2x2 strided-padded),
    # 384:386 bq, 386:452 wva, 452:518 bvb(+ones)

    with tile.TileContext(nc) as tc, ExitStack() as ctx:
        const = ctx.enter_context(tc.tile_pool(name="const", bufs=1))
        big = ctx.enter_context(tc.tile_pool(name="big", bufs=1))
        pss = ctx.enter_context(tc.tile_pool(name="pss", bufs=3, space="PSUM"))
        psav = ctx.enter_context(tc.tile_pool(name="psav", bufs=2, space="PSUM"))
        work = ctx.enter_context(tc.tile_pool(name="work", bufs=6))
        outp = ctx.enter_context(tc.tile_pool(name="outp", bufs=6))

        # ---- input DMA: weights blob, then xt in NT tiles of 1024 cols
        # Two blob tiles so readers only wait for their own piece; service
        # order on the shared transfer engine: xt0, blobV, blobW, xt1..3 --
        # tile 0's v-chunks start as soon as xt0+blobV land.
        blobW = const.tile([128, 386], F32R, tag="blobW")
        blobV = const.tile([128, 132], F32R, tag="blobV")
        xts = []
        t0 = big.tile([128, 1024], F32R, tag="xt0", name="xt0")
        nc.sync.dma_start(t0[:], XT[:, 0:1024].bitcast(F32R))
        xts.append(t0)
        nc.sync.dma_start(blobV[:], WBLOB[:, 386:518].bitcast(F32R))
        nc.sync.dma_start(blobW[:], WBLOB[:, 0:386].bitcast(F32R))
        for c in range(1, NT):
            t = big.tile([128, 1024], F32R, tag=f"xt{c}", name=f"xt{c}")
            nc.sync.dma_start(t[:], XT[:, c * 1024 : (c + 1) * 1024].bitcast(F32R))
            xts.append(t)

        # combined-head weights: output partition p = 64h + 32r + e, so one
        # 128-partition matmul projects q (or packs k) for BOTH heads at once
        wq_comb = blobW[:, 0:128]
        wk_comb = [blobW[:, 128 + 128 * r : 128 + 128 * (r + 1)] for r in range(2)]
        bq_comb = blobW[:, 384:385].bitcast(F32)
        wva = blobV[:, 0:66]
        bvb = blobV[:, 66:132].bitcast(F32)

        # persistent activations (rows 64h+32r+e)
        qt_rep = big.tile([128, S], F32R, tag="qt", name="qt")
        kt_pack = big.tile([128, NG * 128], F32R, tag="kt", name="kt")
        # bf16: the PV matmuls run with bf16 moving data (1 cyc/row, and no
        # f32r-rounding rule on the exp producers)
        vhat = big.tile([128, NJ * 66], BF16, tag="vhat")

        # force the exp_and_others act table (covers identity+exp) up front
        scratch = const.tile([1, 1], F32, tag="scr")
        nc.scalar.activation(scratch[:], blobV[0:1, 0:1].bitcast(F32), AF.Exp)

        # p-state warm-up: ~4.5us of dummy matmuls on zeroed SBUF while the
        # input DMA is in flight, so the real projections start at full PE
        # clock (the ramp needs 3us of contiguous busy)
        zt = const.tile([128, 512], F32, tag="zt")
        nc.vector.memset(zt[:], 0.0)
        ztr = zt.bitcast(F32R)
        zp = pss.tile([128, 1024], F32, tag="s", name="zp")
        for i in range(7):
            nc.tensor.matmul(
                zp[:, 0:512], ztr[:, 0:128], ztr[:, 0:512], start=(i == 0), stop=(i == 6)
            )

        # ---- projection emitters (psum from the pss pool) ----
        def ps_tile(name):
            return pss.tile([128, 1024], F32, tag="s", name=name)

        def v_chunk(j):
            pv = ps_tile(f"pv{j}")
            nc.tensor.matmul(
                pv[:, 0:66],
                xts[j // 8][:, (j % 8) * 128 : (j % 8) * 128 + 128],
                wva,
                start=True,
                stop=True,
            )
            nc.vector.tensor_tensor(
                out=vhat[:, j * 66 : (j + 1) * 66], in0=pv[:, 0:66], in1=bvb, op=ALU.add
            )

        def k_chunk(c):
            # pack kT for groups 4c..4c+3, both heads: partitions 64h+32r+e,
            # col 128g+p
            pk = ps_tile(f"pk{c}")
            xg = xts[c][:].rearrange("d (j p) -> d j p", p=128)
            for r in range(2):
                nc.tensor.matmul(
                    pk[:, 0:512],
                    wk_comb[r],
                    xg[:, r:8:2, :],
                    start=(r == 0),
                    stop=(r == 1),
                )
            # k-mover on DVE so it runs in parallel with the ACT q-movers
            nc.vector.tensor_copy(kt_pack[:, c * 512 : (c + 1) * 512], pk[:, 0:512])

        def q_chunk(n):
            pq = ps_tile(f"pq{n}")
            nc.tensor.matmul(
                pq[:, 0:512],
                wq_comb,
                xts[n // 2][:, (n % 2) * 512 : (n % 2) * 512 + 512],
                start=True,
                stop=True,
            )
            nc.scalar.activation(
                qt_rep[:, n * 512 : (n + 1) * 512],
                pq[:, 0:512],
                AF.Identity,
                bias=bq_comb,
            )

        def proj_tile(c):
            # k/q first: their movers gate the next scores groups, while the
            # v-chunk PE work overlaps those movers
            k_chunk(c)
            q_chunk(2 * c)
            q_chunk(2 * c + 1)
            for j in range(8 * c, 8 * c + 8):
                v_chunk(j)

        # ---- attention ----
        # Decoupled software pipeline: after scores s(g) are emitted, the
        # exp of g-2 is issued (psum WAR horizon = pss bufs 3) and the PV of
        # g-3 (so PV sits ~6 PE slots after its scores: both the ACT and the
        # longer DVE exp latencies are fully hidden).
        last_g = NG - 1
        pending = deque()  # entries: [ps, g, av, h, q0, exp_pt]

        def issue_exp(ent):
            ps, g, av, h, q0, _ = ent
            is_last = h == 1 and q0 == S - 512 and g == last_g
            if is_last:
                # final group: halves on both engines in parallel to shorten
                # the tail chain (PV halves gate on their own half)
                pti = work.tile([128, 1024], I16, tag="pti", name=f"pt{h}_{q0}_{g}")
                nc.scalar.activation(
                    pti[:, 0:512].bitcast(BF16), ps[:, 0:512], AF.Exp
                )
                nc.vector.tensor_scalar(
                    out=pti[:, 512:1024],
                    in0=ps[:, 512:1024],
                    scalar1=SCH_A,
                    scalar2=SCH_B,
                    op0=ALU.mult,
                    op1=ALU.add,
                )
                pt = pti.bitcast(BF16)
            elif g in DVE_GROUPS:
                pti = work.tile([128, 1024], I16, tag="pti", name=f"pt{h}_{q0}_{g}")
                nc.vector.tensor_scalar(
                    out=pti[:],
                    in0=ps[:],
                    scalar1=SCH_A,
                    scalar2=SCH_B,
                    op0=ALU.mult,
                    op1=ALU.add,
                )
                pt = pti.bitcast(BF16)
            else:
                ptf = work.tile([128, 1024], BF16, tag="pt", name=f"pt{h}_{q0}_{g}")
                nc.scalar.activation(ptf[:], ps[:], AF.Exp)
                pt = ptf
            ent[5] = pt

        def issue_pv():
            ent = pending.popleft()
            if ent[5] is None:
                issue_exp(ent)
            _, g, av, h, q0, pt = ent
            for r in range(2):
                j = 2 * g + r
                nc.tensor.matmul(
                    av[:],
                    vhat[:, j * 66 + h * 33 : j * 66 + h * 33 + 33],
                    pt[:, 512 * r : 512 * (r + 1)],
                    start=(g == 0 and r == 0),
                    stop=(g == last_g and r == 1),
                )
            if g == last_g:
                # normalize straight out of psum: reciprocal of the
                # ones-column sums, then broadcast + multiply per slice,
                # store overlapping the next slice's normalize. The final
                # block uses narrower slices to shorten the drain chain.
                nz = 2
                w = 512 // nz
                osb = outp.tile([32, 512], F32, tag="osb", name=f"ob{h}_{q0}")
                rcps = []
                for z in range(nz):
                    rcp = outp.tile([1, w], F32, tag=f"rcp{z}", name=f"rc{h}_{q0}_{z}")
                    nc.vector.reciprocal(rcp[:], av[32:33, w * z : w * (z + 1)])
                    rcps.append(rcp)
                for z in range(nz):
                    cs = slice(w * z, w * (z + 1))
                    bc = outp.tile([32, w], F32, tag=f"bc{z}", name=f"bc{h}_{q0}_{z}")
                    nc.gpsimd.partition_broadcast(bc[:], rcps[z])
                    nc.vector.tensor_mul(osb[:, cs], av[0:32, cs], bc[:])
                    nc.sync.dma_start(OUT[h][:, q0 + w * z : q0 + w * (z + 1)], osb[:, cs])

        def on_group():
            if len(pending) >= 3 and pending[-3][5] is None:
                issue_exp(pending[-3])
            if len(pending) >= 4:
                issue_pv()

        def flush_all():
            for ent in pending:
                if ent[5] is None:
                    issue_exp(ent)
            while pending:
                issue_pv()

        proj_tile(0)
        for h in range(2):
            for i0 in range(NQB):
                q0 = i0 * 512
                av = psav.tile([33, 512], F32, tag="av", name=f"av{h}_{q0}")
                for g in range(NG):
                    ps = pss.tile([128, 1024], F32, tag="s", name=f"s{h}_{q0}_{g}")
                    for r in range(2):
                        off = 64 * h + 32 * r
                        nc.tensor.matmul(
                            ps[:, 512 * r : 512 * (r + 1)],
                            kt_pack[off : off + 32, g * 128 : (g + 1) * 128],
                            qt_rep[off : off + 32, q0 : q0 + 512],
                            start=True,
                            stop=True,
                            tile_position=(off, 0),
                        )
                    pending.append([ps, g, av, h, q0, None])
                    on_group()
                    # interleave remaining xt-tile projections into block 0;
                    # pre-issue pending exps so the 11 psum allocations never
                    # WAR-wait on a not-yet-emitted exp (emission deadlock) —
                    # the PV backlog itself can stay pending.
                    if h == 0 and i0 == 0 and g in (3, 7, 11):
                        c = g // 4 + 1
                        if c < NT:
                            for ent in pending:
                                if ent[5] is None:
                                    issue_exp(ent)
                            proj_tile(c)
        flush_all()

    nc.compile()
    return nc


def _host_prep(x, Wq, bq, Wk, bk, Wv, bv, S):
    """Per-core input maps."""
    in_maps = []
    for c in range(NCORES):
        b, hp = c // 2, c % 2
        h0, h1 = 2 * hp, 2 * hp + 1
        xt = np.ascontiguousarray(x[b].T).astype(np.float32)  # [128, S]
        blob = np.zeros((128, 518), np.float32)
        for i, hh in enumerate((h0, h1)):
            wq_h = Wq[hh * 32 : (hh + 1) * 32, :]  # [32, 128]
            wk_h = Wk[hh * 32 : (hh + 1) * 32, :]
            # combined-head layout: output partition p = 64i + 32r + e
            blob[:, 64 * i : 64 * (i + 1)] = np.tile(wq_h.T, (1, 2))
            for r in range(2):
                off = 128 + 128 * r + 64 * i + 32 * r
                blob[:, off : off + 32] = wk_h.T
            blob[64 * i : 64 * (i + 1), 384] = np.tile(bq[hh * 32 : (hh + 1) * 32], 2)
            blob[:, 386 + 33 * i : 386 + 33 * i + 32] = Wv[hh * 32 : (hh + 1) * 32, :].T
            blob[:, 452 + 33 * i : 452 + 33 * i + 32] = bv[hh * 32 : (hh + 1) * 32][None, :]
            blob[:, 452 + 33 * i + 32] = 1.0
        in_maps.append({"XT": xt, "WBLOB": blob})
    return in_maps


def _unshard(results, S):
    out = np.empty((B, S, D), np.float32)
    for c in range(NCORES):
        b, hp = c // 2, c % 2
        oc = results[c]["OUT"]  # [2, 32, S]
        for hl in range(2):
            hh = 2 * hp + hl
            out[b, :, hh * 32 : (hh + 1) * 32] = oc[hl].T
    return out


def _run_once(args):
    x, Wq, bq, Wk, bk, Wv, bv = args
    S = x.shape[1]
    if S not in _built:
        _built[S] = build_nc(S)
    nc = _built[S]
    in_maps = _host_prep(x, Wq, bq, Wk, bk, Wv, bv, S)
    res = bass_utils.run_bass_kernel_spmd(nc, in_maps, core_ids=list(range(NCORES)))
    return _unshard(res.results, S)


def _subproc_entry(args):
    return _run_once(args)


def kernel(x, Wq, bq, Wk, bk, Wv, bv):
    args = tuple(
        np.asarray(a, dtype=np.float32) for a in (x, Wq, bq, Wk, bk, Wv, bv)
    )
    # The axon/NRT stack occasionally fails a first dispatch with
    # NRT_EXEC_UNIT_UNRECOVERABLE (device auto-recovers). Retry in-process,
    # then in a fresh spawned process (compile caches make that cheap).
    try:
        return _run_once(args)
    except Exception:
        try:
            return _run_once(args)
        except Exception:
            import multiprocessing as mp

            ctx = mp.get_context("spawn")
            with ctx.Pool(1) as pool:
                return pool.apply(_subproc_entry, (args,))



# revision 3
# speedup vs baseline: 1.2287x; 1.2287x over previous
"""Multi-head self-attention (B=4, S=4096, D=128, H=4, no scaling, no mask)
on 8 Trainium2 NeuronCores.

Sharding: 16 (batch, head) pairs over 8 cores -> core c handles batch c//2,
heads 2*(c%2) and 2*(c%2)+1. No cross-core communication.

Per-core algorithm (flash-style, scores never touch DRAM), v3:
  - scores as in v2: psum tile [128, 1024] covers 2 key-chunks of 128 keys
    x 512 queries, via 2 row-tiled matmuls (tile_position (32r,0),
    2-replicated q / packed kT), f32r moving at 1 cyc/row.
  - PV SWAPPED: the exp'd scores pt (bf16) are the STATIONARY operand
    ([128 keys x 128 queries] chunks); vhat [128 keys, 33] is the moving
    one. Output av[128 queries, 33] accumulates over all 32 key chunks in
    psum -> only 33 moving rows per (key-chunk, query-chunk) instead of
    512: ~4x less PE time on PV. Output lands in [query, dim] layout, so
    softmax normalization is a per-partition reciprocal multiply and the
    OUT dma is contiguous.
  - av accumulators for the 4 query-chunks live side by side in one psum
    bank [128, 132]; a dummy zero matmul (start=True over all 132 cols)
    opens the bank, all real PV matmuls accumulate with start=False.
  - exp split across THREE engines (ACT real Exp -> bf16; DVE and POOL
    Schraudolph fast-exp: int16(A*s+B) bitcast to bf16, ~3% sawtooth
    error, within the 2e-2 tolerance; denominators stay consistent
    because the ones-column sums the same approximated values).
  - bk is dropped entirely (softmax invariant); bv is kept in vhat via
    the DVE bias-add as in v2.
  - v projection moving operand (wv) is bf16 -> 1 cyc/row (f32r moving
    under 256 rows would cost 4 cyc/row).
  - software pipeline: scores(g) emitted; exp(g-1) issued; PV(g-3)
    issued. psum scores pool bufs=3 (6 banks) + av pool bufs=2 (2 banks).
Host gathers OUT [2, S, 32] per core into the full (B, S, D) output.
"""

import sys

for _p in ("/opt/trn_rl_repo", "/root/.axon_site/_ro/trn_rl_repo"):
    if _p not in sys.path:
        sys.path.append(_p)

import numpy as np
from collections import deque
from contextlib import ExitStack

import concourse.bass as bass
import concourse.bacc as bacc
import concourse.mybir as mybir
import concourse.tile as tile
from concourse import bass_utils

F32 = mybir.dt.float32
F32R = mybir.dt.float32r
I32 = mybir.dt.int32
I16 = mybir.dt.int16
BF16 = mybir.dt.bfloat16
AF = mybir.ActivationFunctionType
ALU = mybir.AluOpType

B, D, H, HD = 4, 128, 4, 32
NCORES = 8

# Schraudolph fast-exp in bf16 bit-space: exp(x) ~= bitcast_bf16(int16(A*x+B))
# (bf16 = top 16 bits of f32, so the fp32 constants scale by 2^-16)
LOG2E = 1.4426950408889634
SCH_A = float(np.float32(2.0**7 * LOG2E))
SCH_C = 486411.0 / 2.0**16
SCH_B = float(np.float32(127.0 * 2.0**7 - SCH_C))

# exp engine per group slot (A=ACT real exp, D=DVE fast-exp, P=POOL fast-exp)
# steady-state blocks: ACT 7/16, DVE 5/16, POOL 4/16
EXP_PAT = "ADPADAPADPADAPDA"
# block 0 carries the projection work for xt tiles 1..3 (DVE: vhat bias
# adds; ACT: q bias + kt movers) -> balanced 6/5/5 split there
EXP_PAT_B0 = "ADPADPADPADPADPA"

_built = {}


def build_nc(S):
    """Build + compile the per-core program (identical across cores)."""
    NJ = S // 128   # 128-key chunks
    NG = NJ // 2    # 2-chunk groups per block
    NQB = S // 512  # 512-query blocks per head
    NT = S // 1024  # xt DMA tiles

    nc = bacc.Bacc("TRN2", target_bir_lowering=False, debug=False)

    XT = nc.dram_tensor("XT", [128, S], F32, kind="ExternalInput").ap()
    WBLOB = nc.dram_tensor("WBLOB", [128, 518], F32, kind="ExternalInput").ap()
    WVB = nc.dram_tensor("WVB", [128, 66], BF16, kind="ExternalInput").ap()
    OUT = nc.dram_tensor("OUT", [2, S, 32], F32, kind="ExternalOutput").ap()
    # WBLOB cols: 0:128 wq (2-replicated), 128:384 wk (2x2 strided-padded),
    # 384:386 bq, 386:452 unused (legacy wva), 452:518 bvb(+ones)

    with tile.TileContext(nc) as tc, ExitStack() as ctx:
        const = ctx.enter_context(tc.tile_pool(name="const", bufs=1))
        big = ctx.enter_context(tc.tile_pool(name="big", bufs=1))
        pss = ctx.enter_context(tc.tile_pool(name="pss", bufs=3, space="PSUM"))
        psav = ctx.enter_context(tc.tile_pool(name="psav", bufs=2, space="PSUM"))
        work = ctx.enter_context(tc.tile_pool(name="work", bufs=6))
        outp = ctx.enter_context(tc.tile_pool(name="outp", bufs=8))

        # ---- input DMA: weights blob, then xt in NT tiles of 1024 cols
        # Service order on the shared transfer engine: xt0, wvb, blobV,
        # blobW, xt1..3 -- tile 0's v-chunks start as soon as xt0+wvb land.
        blobW = const.tile([128, 386], F32R, tag="blobW")
        blobV = const.tile([128, 132], F32R, tag="blobV")
        wvb = const.tile([128, 66], BF16, tag="wvb")
        xts = []
        t0 = big.tile([128, 1024], F32R, tag="xt0", name="xt0")
        nc.sync.dma_start(t0[:], XT[:, 0:1024].bitcast(F32R))
        xts.append(t0)
        nc.sync.dma_start(wvb[:], WVB[:, :])
        nc.sync.dma_start(blobV[:], WBLOB[:, 386:518].bitcast(F32R))
        nc.sync.dma_start(blobW[:], WBLOB[:, 0:386].bitcast(F32R))
        for c in range(1, NT):
            t = big.tile([128, 1024], F32R, tag=f"xt{c}", name=f"xt{c}")
            nc.sync.dma_start(t[:], XT[:, c * 1024 : (c + 1) * 1024].bitcast(F32R))
            xts.append(t)

        # combined-head weights: output partition p = 64h + 32r + e, so one
        # 128-partition matmul projects q (or packs k) for BOTH heads at once
        wq_comb = blobW[:, 0:128]
        wk_comb = [blobW[:, 128 + 128 * r : 128 + 128 * (r + 1)] for r in range(2)]
        bq_comb = blobW[:, 384:385].bitcast(F32)
        bvb = blobV[:, 66:132].bitcast(F32)

        # persistent activations (rows 64h+32r+e)
        qt_rep = big.tile([128, S], F32R, tag="qt", name="qt")
        kt_pack = big.tile([128, NG * 128], F32R, tag="kt", name="kt")
        # bf16: PV runs fully in bf16 (stationary pt, moving vhat)
        vhat = big.tile([128, NJ * 66], BF16, tag="vhat")

        # bf16 zeros for the av-bank-opening dummy matmul
        zbf = const.tile([128, 132], BF16, tag="zbf")
        nc.vector.memset(zbf[:], 0.0)

        # force the exp_and_others act table (covers identity+exp) up front
        scratch = const.tile([1, 1], F32, tag="scr")
        nc.scalar.activation(scratch[:], blobV[0:1, 0:1].bitcast(F32), AF.Exp)

        # p-state warm-up: ~4.5us of dummy matmuls on zeroed SBUF while the
        # input DMA is in flight, so the real projections start at full PE
        # clock (the ramp needs 3us of contiguous busy)
        zt = const.tile([128, 512], F32, tag="zt")
        nc.vector.memset(zt[:], 0.0)
        ztr = zt.bitcast(F32R)
        zp = pss.tile([128, 1024], F32, tag="s", name="zp")
        for i in range(7):
            nc.tensor.matmul(
                zp[:, 0:512], ztr[:, 0:128], ztr[:, 0:512], start=(i == 0), stop=(i == 6)
            )

        # ---- projection emitters (psum from the pss pool) ----
        def ps_tile(name):
            return pss.tile([128, 1024], F32, tag="s", name=name)

        def v_chunk(j):
            pv = ps_tile(f"pv{j}")
            nc.tensor.matmul(
                pv[:, 0:66],
                xts[j // 8][:, (j % 8) * 128 : (j % 8) * 128 + 128],
                wvb[:],
                start=True,
                stop=True,
            )
            nc.vector.tensor_tensor(
                out=vhat[:, j * 66 : (j + 1) * 66], in0=pv[:, 0:66], in1=bvb, op=ALU.add
            )

        def k_chunk(c):
            # pack kT for groups 4c..4c+3, both heads: partitions 64h+32r+e,
            # col 128g+p
            pk = ps_tile(f"pk{c}")
            xg = xts[c][:].rearrange("d (j p) -> d j p", p=128)
            for r in range(2):
                nc.tensor.matmul(
                    pk[:, 0:512],
                    wk_comb[r],
                    xg[:, r:8:2, :],
                    start=(r == 0),
                    stop=(r == 1),
                )
            # k-mover on ACT so DVE keeps room for the vhat bias adds
            nc.scalar.activation(
                kt_pack[:, c * 512 : (c + 1) * 512].bitcast(F32),
                pk[:, 0:512],
                AF.Identity,
            )

        def q_chunk(n):
            pq = ps_tile(f"pq{n}")
            nc.tensor.matmul(
                pq[:, 0:512],
                wq_comb,
                xts[n // 2][:, (n % 2) * 512 : (n % 2) * 512 + 512],
                start=True,
                stop=True,
            )
            nc.scalar.activation(
                qt_rep[:, n * 512 : (n + 1) * 512].bitcast(F32),
                pq[:, 0:512],
                AF.Identity,
                bias=bq_comb,
            )

        def proj_tile(c):
            # k/q first: their movers gate the next scores groups, while the
            # v-chunk PE work overlaps those movers
            k_chunk(c)
            q_chunk(2 * c)
            q_chunk(2 * c + 1)
            for j in range(8 * c, 8 * c + 8):
                v_chunk(j)

        # ---- attention ----
        # Decoupled software pipeline: after scores s(g) are emitted, the
        # exp of g-1 is issued and the PV of g-3.
        last_g = NG - 1
        pending = deque()  # entries: [ps, g, av, h, q0, exp_pt]

        def issue_exp(ent, in_b0):
            ps, g, av, h, q0, _ = ent
            eng = (EXP_PAT_B0 if in_b0 else EXP_PAT)[g]
            if eng == "A":
                ptf = work.tile([128, 1024], BF16, tag="pt", name=f"pt{h}_{q0}_{g}")
                nc.scalar.activation(ptf[:], ps[:], AF.Exp)
                pt = ptf
            else:
                pti = work.tile([128, 1024], I16, tag="pti", name=f"pt{h}_{q0}_{g}")
                e = nc.vector if eng == "D" else nc.gpsimd
                e.tensor_scalar(
                    out=pti[:],
                    in0=ps[:],
                    scalar1=SCH_A,
                    scalar2=SCH_B,
                    op0=ALU.mult,
                    op1=ALU.add,
                )
                pt = pti.bitcast(BF16)
            ent[5] = pt

        def issue_pv():
            ent = pending.popleft()
            if ent[5] is None:
                issue_exp(ent, False)
            _, g, av, h, q0, pt = ent
            for r in range(2):
                j = 2 * g + r
                vs = vhat[:, j * 66 + h * 33 : j * 66 + h * 33 + 33]
                for qc in range(4):
                    nc.tensor.matmul(
                        av[:, qc * 33 : qc * 33 + 33],
                        pt[:, 512 * r + 128 * qc : 512 * r + 128 * qc + 128],
                        vs,
                        start=False,
                        stop=(g == last_g and r == 1 and qc == 3),
                        skip_group_check=True,
                    )
            if g == last_g:
                # normalize straight out of psum: per-partition reciprocal of
                # the ones-columns, then one tensor_scalar multiply per
                # 128-query chunk; single contiguous DMA for the block.
                rcp = outp.tile([128, 4], F32, tag="rcp", name=f"rc{h}_{q0}")
                nc.vector.reciprocal(rcp[:], av[:, 32:132:33])
                osb = outp.tile([128, 128], F32, tag="osb", name=f"ob{h}_{q0}")
                for qc in range(4):
                    nc.vector.tensor_scalar(
                        out=osb[:, qc * 32 : qc * 32 + 32],
                        in0=av[:, qc * 33 : qc * 33 + 32],
                        scalar1=rcp[:, qc : qc + 1],
                        scalar2=None,
                        op0=ALU.mult,
                    )
                nc.sync.dma_start(
                    OUT[h, q0 : q0 + 512, :].rearrange("(c p) d -> p c d", c=4),
                    osb[:].rearrange("p (c d) -> p c d", c=4),
                )

        def on_group(in_b0):
            if len(pending) >= 2 and pending[-2][5] is None:
                issue_exp(pending[-2], in_b0)
            if len(pending) >= 4:
                issue_pv()

        def flush_all():
            for ent in pending:
                if ent[5] is None:
                    issue_exp(ent, False)
            while pending:
                issue_pv()

        proj_tile(0)
        for h in range(2):
            for i0 in range(NQB):
                q0 = i0 * 512
                in_b0 = h == 0 and i0 == 0
                av = psav.tile([128, 512], F32, tag="av", name=f"av{h}_{q0}")
                # open the accumulation bank: zeros over all 132 cols
                nc.tensor.matmul(
                    av[:, 0:132],
                    zbf[:, 0:128],
                    zbf[:, 0:132],
                    start=True,
                    stop=False,
                    skip_group_check=True,
                )
                for g in range(NG):
                    ps = pss.tile([128, 1024], F32, tag="s", name=f"s{h}_{q0}_{g}")
                    for r in range(2):
                        off = 64 * h + 32 * r
                        nc.tensor.matmul(
                            ps[:, 512 * r : 512 * (r + 1)],
                            kt_pack[off : off + 32, g * 128 : (g + 1) * 128],
                            qt_rep[off : off + 32, q0 : q0 + 512],
                            start=True,
                            stop=True,
                            tile_position=(off, 0),
                        )
                    pending.append([ps, g, av, h, q0, None])
                    on_group(in_b0)
                    # interleave remaining xt-tile projections into block 0;
                    # pre-issue pending exps so the 11 psum allocations never
                    # WAR-wait on a not-yet-emitted exp (emission deadlock) —
                    # the PV backlog itself can stay pending.
                    if in_b0 and g in (3, 7, 11):
                        c = g // 4 + 1
                        if c < NT:
                            for ent in pending:
                                if ent[5] is None:
                                    issue_exp(ent, True)
                            proj_tile(c)
        flush_all()

    nc.compile()
    return nc


def _host_prep(x, Wq, bq, Wk, bk, Wv, bv, S):
    """Per-core input maps."""
    try:
        import ml_dtypes

        bf16 = np.dtype(ml_dtypes.bfloat16)
    except ImportError:  # pragma: no cover
        bf16 = None
    in_maps = []
    for c in range(NCORES):
        b, hp = c // 2, c % 2
        h0, h1 = 2 * hp, 2 * hp + 1
        xt = np.ascontiguousarray(x[b].T).astype(np.float32)  # [128, S]
        blob = np.zeros((128, 518), np.float32)
        wvb = np.zeros((128, 66), np.float32)
        for i, hh in enumerate((h0, h1)):
            wq_h = Wq[hh * 32 : (hh + 1) * 32, :]  # [32, 128]
            wk_h = Wk[hh * 32 : (hh + 1) * 32, :]
            # combined-head layout: output partition p = 64i + 32r + e
            blob[:, 64 * i : 64 * (i + 1)] = np.tile(wq_h.T, (1, 2))
            for r in range(2):
                off = 128 + 128 * r + 64 * i + 32 * r
                blob[:, off : off + 32] = wk_h.T
            blob[64 * i : 64 * (i + 1), 384] = np.tile(bq[hh * 32 : (hh + 1) * 32], 2)
            wvb[:, 33 * i : 33 * i + 32] = Wv[hh * 32 : (hh + 1) * 32, :].T
            blob[:, 452 + 33 * i : 452 + 33 * i + 32] = bv[hh * 32 : (hh + 1) * 32][None, :]
            blob[:, 452 + 33 * i + 32] = 1.0
        wvb16 = wvb.astype(bf16) if bf16 is not None else wvb.astype(np.float32)
        in_maps.append({"XT": xt, "WBLOB": blob, "WVB": wvb16})
    return in_maps


def _unshard(results, S):
    out = np.empty((B, S, D), np.float32)
    for c in range(NCORES):
        b, hp = c // 2, c % 2
        oc = results[c]["OUT"]  # [2, S, 32]
        for hl in range(2):
            hh = 2 * hp + hl
            out[b, :, hh * 32 : (hh + 1) * 32] = oc[hl]
    return out


def _run_once(args):
    x, Wq, bq, Wk, bk, Wv, bv = args
    S = x.shape[1]
    if S not in _built:
        _built[S] = build_nc(S)
    nc = _built[S]
    in_maps = _host_prep(x, Wq, bq, Wk, bk, Wv, bv, S)
    res = bass_utils.run_bass_kernel_spmd(nc, in_maps, core_ids=list(range(NCORES)))
    return _unshard(res.results, S)


def _subproc_entry(args):
    return _run_once(args)


def kernel(x, Wq, bq, Wk, bk, Wv, bv):
    args = tuple(
        np.asarray(a, dtype=np.float32) for a in (x, Wq, bq, Wk, bk, Wv, bv)
    )
    # The axon/NRT stack occasionally fails a first dispatch with
    # NRT_EXEC_UNIT_UNRECOVERABLE (device auto-recovers). Retry in-process,
    # then in a fresh spawned process (compile caches make that cheap).
    try:
        return _run_once(args)
    except Exception:
        try:
            return _run_once(args)
        except Exception:
            import multiprocessing as mp

            ctx = mp.get_context("spawn")
            with ctx.Pool(1) as pool:
                return pool.apply(_subproc_entry, (args,))


# revision 7
# speedup vs baseline: 1.2290x; 1.0003x over previous
"""Multi-head self-attention (B=4, S=4096, D=128, H=4, no scaling, no mask)
on 8 Trainium2 NeuronCores.

Sharding: 16 (batch, head) pairs over 8 cores -> core c handles batch c//2,
heads 2*(c%2) and 2*(c%2)+1. No cross-core communication.

Per-core algorithm (flash-style, scores never touch DRAM), v3:
  - scores as in v2: psum tile [128, 1024] covers 2 key-chunks of 128 keys
    x 512 queries, via 2 row-tiled matmuls (tile_position (32r,0),
    2-replicated q / packed kT), f32r moving at 1 cyc/row.
  - PV SWAPPED: the exp'd scores pt (bf16) are the STATIONARY operand
    ([128 keys x 128 queries] chunks); vhat [128 keys, 33] is the moving
    one. Output av[128 queries, 33] accumulates over all 32 key chunks in
    psum -> only 33 moving rows per (key-chunk, query-chunk) instead of
    512: ~4x less PE time on PV. Output lands in [query, dim] layout, so
    softmax normalization is a per-partition reciprocal multiply and the
    OUT dma is contiguous.
  - av accumulators for the 4 query-chunks live side by side in one psum
    bank [128, 132]; a dummy zero matmul (start=True over all 132 cols)
    opens the bank, all real PV matmuls accumulate with start=False.
  - exp split across ACT (real Exp -> bf16) and DVE (Schraudolph
    fast-exp: int16(A*s+B) bitcast to bf16, ~3% sawtooth error, within
    the 2e-2 tolerance; denominators stay consistent because the
    ones-column sums the same approximated values). Pool/GPSIMD cannot
    access PSUM so it cannot help with the exp.
  - bk is dropped entirely (softmax invariant); bv is kept in vhat via
    the DVE bias-add as in v2.
  - software pipeline: scores(g) emitted; exp(g-1) issued; PV(g-3)
    issued. psum scores pool bufs=3 (6 banks) + av pool bufs=2 (2 banks).
Host gathers OUT [2, S, 32] per core into the full (B, S, D) output.
"""

import sys

for _p in ("/opt/trn_rl_repo", "/root/.axon_site/_ro/trn_rl_repo"):
    if _p not in sys.path:
        sys.path.append(_p)

import numpy as np
from collections import deque
from contextlib import ExitStack

import concourse.bass as bass
import concourse.bacc as bacc
import concourse.mybir as mybir
import concourse.tile as tile
from concourse import bass_utils

F32 = mybir.dt.float32
F32R = mybir.dt.float32r
I32 = mybir.dt.int32
I16 = mybir.dt.int16
BF16 = mybir.dt.bfloat16
AF = mybir.ActivationFunctionType
ALU = mybir.AluOpType

B, D, H, HD = 4, 128, 4, 32
NCORES = 8

# Schraudolph fast-exp in bf16 bit-space: exp(x) ~= bitcast_bf16(int16(A*x+B))
# (bf16 = top 16 bits of f32, so the fp32 constants scale by 2^-16)
LOG2E = 1.4426950408889634
SCH_A = float(np.float32(2.0**7 * LOG2E))
SCH_C = 486411.0 / 2.0**16
SCH_B = float(np.float32(127.0 * 2.0**7 - SCH_C))

# exp engine per group slot (A=ACT real exp, D=DVE Schraudolph fast-exp).
# GPSIMD/Pool cannot access PSUM, so only ACT and DVE can evacuate scores.
# steady-state blocks: ACT 9/16, DVE 7/16 (ACT is a bit faster per group)
EXP_PAT = "AADADADADADADADA"
# block 0 carries the projection work for xt tiles 1..3 (DVE: vhat bias
# adds; ACT: q bias + kt movers) -> even split there
EXP_PAT_B0 = "ADADADADADADADAD"

_built = {}


def build_nc(S):
    """Build + compile the per-core program (identical across cores)."""
    NJ = S // 128   # 128-key chunks
    NG = NJ // 2    # 2-chunk groups per block
    NQB = S // 512  # 512-query blocks per head
    NT = S // 1024  # xt DMA tiles

    nc = bacc.Bacc("TRN2", target_bir_lowering=False, debug=False)

    XT = nc.dram_tensor("XT", [128, S], F32, kind="ExternalInput").ap()
    WBLOB = nc.dram_tensor("WBLOB", [128, 518], F32, kind="ExternalInput").ap()
    OUT = nc.dram_tensor("OUT", [2, S, 32], F32, kind="ExternalOutput").ap()
    # WBLOB cols: 0:128 wq (2-replicated), 128:384 wk (2x2 strided-padded),
    # 384:386 bq, 386:452 wva, 452:518 bvb(+ones)

    with tile.TileContext(nc) as tc, ExitStack() as ctx:
        const = ctx.enter_context(tc.tile_pool(name="const", bufs=1))
        big = ctx.enter_context(tc.tile_pool(name="big", bufs=1))
        pss = ctx.enter_context(tc.tile_pool(name="pss", bufs=3, space="PSUM"))
        psav = ctx.enter_context(tc.tile_pool(name="psav", bufs=2, space="PSUM"))
        work = ctx.enter_context(tc.tile_pool(name="work", bufs=6))
        outp = ctx.enter_context(tc.tile_pool(name="outp", bufs=8))

        # ---- input DMA: weights blob, then xt in NT tiles of 1024 cols
        # Service order on the shared transfer engine: xt0, wvb, blobV,
        # blobW, xt1..3 -- tile 0's v-chunks start as soon as xt0+wvb land.
        blobW = const.tile([128, 386], F32R, tag="blobW")
        blobV = const.tile([128, 132], F32R, tag="blobV")
        xts = []
        t0 = big.tile([128, 1024], F32R, tag="xt0", name="xt0")
        nc.sync.dma_start(t0[:], XT[:, 0:1024].bitcast(F32R))
        xts.append(t0)
        nc.sync.dma_start(blobV[:], WBLOB[:, 386:518].bitcast(F32R))
        nc.sync.dma_start(blobW[:], WBLOB[:, 0:386].bitcast(F32R))
        for c in range(1, NT):
            t = big.tile([128, 1024], F32R, tag=f"xt{c}", name=f"xt{c}")
            nc.sync.dma_start(t[:], XT[:, c * 1024 : (c + 1) * 1024].bitcast(F32R))
            xts.append(t)

        # combined-head weights: output partition p = 64h + 32r + e, so one
        # 128-partition matmul projects q (or packs k) for BOTH heads at once
        wq_comb = blobW[:, 0:128]
        wk_comb = [blobW[:, 128 + 128 * r : 128 + 128 * (r + 1)] for r in range(2)]
        bq_comb = blobW[:, 384:385].bitcast(F32)
        wva = blobV[:, 0:66]
        bvb = blobV[:, 66:132].bitcast(F32)

        # persistent activations (rows 64h+32r+e)
        qt_rep = big.tile([128, S], F32R, tag="qt", name="qt")
        kt_pack = big.tile([128, NG * 128], F32R, tag="kt", name="kt")
        # bf16: PV runs fully in bf16 (stationary pt, moving vhat)
        vhat = big.tile([128, NJ * 66], BF16, tag="vhat")

        # bf16 zeros for the av-bank-opening dummy matmul
        zbf = const.tile([128, 132], BF16, tag="zbf")
        nc.vector.memset(zbf[:], 0.0)

        # force the exp_and_others act table (covers identity+exp) up front
        scratch = const.tile([1, 1], F32, tag="scr")
        nc.scalar.activation(scratch[:], blobV[0:1, 0:1].bitcast(F32), AF.Exp)

        # p-state warm-up: ~4.5us of dummy matmuls on zeroed SBUF while the
        # input DMA is in flight, so the real projections start at full PE
        # clock (the ramp needs 3us of contiguous busy)
        zt = const.tile([128, 512], F32, tag="zt")
        nc.vector.memset(zt[:], 0.0)
        ztr = zt.bitcast(F32R)
        zp = pss.tile([128, 1024], F32, tag="s", name="zp")
        for i in range(7):
            nc.tensor.matmul(
                zp[:, 0:512], ztr[:, 0:128], ztr[:, 0:512], start=(i == 0), stop=(i == 6)
            )

        # ---- projection emitters (psum from the pss pool) ----
        def ps_tile(name):
            return pss.tile([128, 1024], F32, tag="s", name=name)

        def v_chunk(j):
            pv = ps_tile(f"pv{j}")
            nc.tensor.matmul(
                pv[:, 0:66],
                xts[j // 8][:, (j % 8) * 128 : (j % 8) * 128 + 128],
                wva,
                start=True,
                stop=True,
            )
            nc.vector.tensor_tensor(
                out=vhat[:, j * 66 : (j + 1) * 66], in0=pv[:, 0:66], in1=bvb, op=ALU.add
            )

        def k_chunk(c):
            # pack kT for groups 4c..4c+3, both heads: partitions 64h+32r+e,
            # col 128g+p
            pk = ps_tile(f"pk{c}")
            xg = xts[c][:].rearrange("d (j p) -> d j p", p=128)
            for r in range(2):
                nc.tensor.matmul(
                    pk[:, 0:512],
                    wk_comb[r],
                    xg[:, r:8:2, :],
                    start=(r == 0),
                    stop=(r == 1),
                )
            # k-mover on ACT so DVE keeps room for the vhat bias adds
            nc.scalar.activation(
                kt_pack[:, c * 512 : (c + 1) * 512],
                pk[:, 0:512],
                AF.Identity,
            )

        def q_chunk(n):
            pq = ps_tile(f"pq{n}")
            nc.tensor.matmul(
                pq[:, 0:512],
                wq_comb,
                xts[n // 2][:, (n % 2) * 512 : (n % 2) * 512 + 512],
                start=True,
                stop=True,
            )
            nc.scalar.activation(
                qt_rep[:, n * 512 : (n + 1) * 512],
                pq[:, 0:512],
                AF.Identity,
                bias=bq_comb,
            )

        def proj_tile(c):
            # k/q first: their movers gate the next scores groups, while the
            # v-chunk PE work overlaps those movers
            k_chunk(c)
            q_chunk(2 * c)
            q_chunk(2 * c + 1)
            for j in range(8 * c, 8 * c + 8):
                v_chunk(j)

        # ---- attention ----
        # Decoupled software pipeline: after scores s(g) are emitted, the
        # exp of g-1 is issued and the PV of g-3.
        last_g = NG - 1
        pending = deque()  # entries: [ps, g, av, h, q0, exp_pt]

        def issue_exp(ent, in_b0):
            ps, g, av, h, q0, _ = ent
            eng = (EXP_PAT_B0 if in_b0 else EXP_PAT)[g]
            if eng == "A":
                ptf = work.tile([128, 1024], BF16, tag="pt", name=f"pt{h}_{q0}_{g}")
                nc.scalar.activation(ptf[:], ps[:], AF.Exp)
                pt = ptf
            else:
                pti = work.tile([128, 1024], I16, tag="pti", name=f"pt{h}_{q0}_{g}")
                nc.vector.tensor_scalar(
                    out=pti[:],
                    in0=ps[:],
                    scalar1=SCH_A,
                    scalar2=SCH_B,
                    op0=ALU.mult,
                    op1=ALU.add,
                )
                pt = pti.bitcast(BF16)
            ent[5] = pt

        def issue_pv():
            ent = pending.popleft()
            if ent[5] is None:
                issue_exp(ent, False)
            _, g, av, h, q0, pt = ent
            for r in range(2):
                j = 2 * g + r
                vs = vhat[:, j * 66 + h * 33 : j * 66 + h * 33 + 33]
                for qc in range(4):
                    nc.tensor.matmul(
                        av[:, qc * 33 : qc * 33 + 33],
                        pt[:, 512 * r + 128 * qc : 512 * r + 128 * qc + 128],
                        vs,
                        start=False,
                        stop=(g == last_g and r == 1 and qc == 3),
                        skip_group_check=True,
                    )
            if g == last_g:
                # normalize straight out of psum: per-partition reciprocal of
                # the ones-columns, then one tensor_scalar multiply per
                # 128-query chunk; single contiguous DMA for the block.
                rcp = outp.tile([128, 4], F32, tag="rcp", name=f"rc{h}_{q0}")
                nc.vector.reciprocal(rcp[:], av[:, 32:132:33])
                osb = outp.tile([128, 128], F32, tag="osb", name=f"ob{h}_{q0}")
                for qc in range(4):
                    nc.vector.tensor_scalar(
                        out=osb[:, qc * 32 : qc * 32 + 32],
                        in0=av[:, qc * 33 : qc * 33 + 32],
                        scalar1=rcp[:, qc : qc + 1],
                        scalar2=None,
                        op0=ALU.mult,
                    )
                nc.sync.dma_start(
                    OUT[h, q0 : q0 + 512, :].rearrange("(c p) d -> p c d", c=4),
                    osb[:].rearrange("p (c d) -> p c d", c=4),
                )

        def on_group(in_b0):
            if len(pending) >= 2 and pending[-2][5] is None:
                issue_exp(pending[-2], in_b0)
            if len(pending) >= 4:
                issue_pv()

        def flush_all():
            for ent in pending:
                if ent[5] is None:
                    issue_exp(ent, False)
            while pending:
                issue_pv()

        proj_tile(0)
        for h in range(2):
            for i0 in range(NQB):
                q0 = i0 * 512
                in_b0 = h == 0 and i0 == 0
                av = psav.tile([128, 512], F32, tag="av", name=f"av{h}_{q0}")
                # open the accumulation bank: zeros over all 132 cols
                nc.tensor.matmul(
                    av[:, 0:132],
                    zbf[:, 0:128],
                    zbf[:, 0:132],
                    start=True,
                    stop=False,
                    skip_group_check=True,
                )
                for g in range(NG):
                    ps = pss.tile([128, 1024], F32, tag="s", name=f"s{h}_{q0}_{g}")
                    for r in range(2):
                        off = 64 * h + 32 * r
                        nc.tensor.matmul(
                            ps[:, 512 * r : 512 * (r + 1)],
                            kt_pack[off : off + 32, g * 128 : (g + 1) * 128],
                            qt_rep[off : off + 32, q0 : q0 + 512],
                            start=True,
                            stop=True,
                            tile_position=(off, 0),
                        )
                    pending.append([ps, g, av, h, q0, None])
                    on_group(in_b0)
                    # interleave remaining xt-tile projections into block 0;
                    # pre-issue pending exps so the 11 psum allocations never
                    # WAR-wait on a not-yet-emitted exp (emission deadlock) —
                    # the PV backlog itself can stay pending.
                    if in_b0 and g in (3, 7, 11):
                        c = g // 4 + 1
                        if c < NT:
                            for ent in pending:
                                if ent[5] is None:
                                    issue_exp(ent, True)
                            proj_tile(c)
        flush_all()

    nc.compile()
    return nc


def _host_prep(x, Wq, bq, Wk, bk, Wv, bv, S):
    """Per-core input maps."""
    in_maps = []
    for c in range(NCORES):
        b, hp = c // 2, c % 2
        h0, h1 = 2 * hp, 2 * hp + 1
        xt = np.ascontiguousarray(x[b].T).astype(np.float32)  # [128, S]
        blob = np.zeros((128, 518), np.float32)
        for i, hh in enumerate((h0, h1)):
            wq_h = Wq[hh * 32 : (hh + 1) * 32, :]  # [32, 128]
            wk_h = Wk[hh * 32 : (hh + 1) * 32, :]
            # combined-head layout: output partition p = 64i + 32r + e
            blob[:, 64 * i : 64 * (i + 1)] = np.tile(wq_h.T, (1, 2))
            for r in range(2):
                off = 128 + 128 * r + 64 * i + 32 * r
                blob[:, off : off + 32] = wk_h.T
            blob[64 * i : 64 * (i + 1), 384] = np.tile(bq[hh * 32 : (hh + 1) * 32], 2)
            blob[:, 386 + 33 * i : 386 + 33 * i + 32] = Wv[hh * 32 : (hh + 1) * 32, :].T
            blob[:, 452 + 33 * i : 452 + 33 * i + 32] = bv[hh * 32 : (hh + 1) * 32][None, :]
            blob[:, 452 + 33 * i + 32] = 1.0
        in_maps.append({"XT": xt, "WBLOB": blob})
    return in_maps


def _unshard(results, S):
    out = np.empty((B, S, D), np.float32)
    for c in range(NCORES):
        b, hp = c // 2, c % 2
        oc = results[c]["OUT"]  # [2, S, 32]
        for hl in range(2):
            hh = 2 * hp + hl
            out[b, :, hh * 32 : (hh + 1) * 32] = oc[hl]
    return out


def _run_once(args):
    x, Wq, bq, Wk, bk, Wv, bv = args
    S = x.shape[1]
    if S not in _built:
        _built[S] = build_nc(S)
    nc = _built[S]
    in_maps = _host_prep(x, Wq, bq, Wk, bk, Wv, bv, S)
    res = bass_utils.run_bass_kernel_spmd(nc, in_maps, core_ids=list(range(NCORES)))
    return _unshard(res.results, S)


def _subproc_entry(args):
    return _run_once(args)


def kernel(x, Wq, bq, Wk, bk, Wv, bv):
    args = tuple(
        np.asarray(a, dtype=np.float32) for a in (x, Wq, bq, Wk, bk, Wv, bv)
    )
    # The axon/NRT stack occasionally fails a first dispatch with
    # NRT_EXEC_UNIT_UNRECOVERABLE (device auto-recovers). Retry in-process,
    # then in a fresh spawned process (compile caches make that cheap).
    try:
        return _run_once(args)
    except Exception:
        try:
            return _run_once(args)
        except Exception:
            import multiprocessing as mp

            ctx = mp.get_context("spawn")
            with ctx.Pool(1) as pool:
                return pool.apply(_subproc_entry, (args,))
